# revision 22
# baseline (speedup 1.0000x reference)
"""Trainium2 Bass kernel for MoE router (BaseRouter): 8-core data-parallel.

Reference computation (per token): router MLP (Linear-ReLU-Linear) -> softmax
-> top-2 -> dispatch/combine one-hot tensors [N, E, CAPACITY] + aux load loss.

Sharding: tokens (B*S = 4096) split 512/core across 8 cores; router weights
replicated. The dispatch/combine capacity index is always 0 for a token's
top-1 expert and `appeared[e]` (0/1) for its top-2 expert, where appeared[e]
says whether ANY token globally picked e as top-1.

The kernel is output-write bound (2 x 25 MiB of mostly-zero rows per core),
so the structure optimizes time-to-first-write and write bandwidth:
 - outputs are staged as half-rows (4 experts x 1536 capacity) in SBUF:
   24 KiB DMA packets, double-buffered, only the 8 columns [e, 0:2] are
   ever rewritten after a one-time memset; dispatch rides the sync HWDGE
   queue and combine the scalar one;
 - compute is pipelined per token segment so the first row DMA issues after
   ~1/5 of the MLP; later blocks share matmul weight loads (wider moving
   dim) since the write wall, not the PE, paces them;
 - rows are written assuming appeared[e] == 1 (true unless some expert is
   nobody's top-1); the 8-core AllReduce (expert counts fused with the
   aux-loss prob sums) runs off the critical path, overlapped with the
   remaining output writes. The degenerate appeared[e] == 0 case is
   patched during the host-side unshard from the (always exact) probs.
"""

import sys

if "/opt/trn_rl_repo" not in sys.path:
    sys.path.insert(0, "/opt/trn_rl_repo")

import numpy as np

import concourse.bass as bass  # noqa: F401  (engine types referenced via nc)
import concourse.mybir as mybir
import concourse.tile as tile
from concourse import bacc
from concourse.bass_utils import run_bass_kernel_spmd
from concourse.masks import make_identity

B, S, H, E, TOPK = 2, 2048, 1024, 8, 2
CAPACITY = 1536
N = B * S                  # 4096 tokens
NCORES = 8
NT = N // NCORES           # 512 tokens per core
P = 128                    # SBUF partitions
TBLK = NT // P             # 4 token tiles per core
KC = H // P                # 8 contraction chunks
DT = mybir.dt.float32
# (start_block, n_blocks) compute segments: early blocks solo to open the
# write pipeline ASAP, later blocks fused (weight loads amortized over a
# wider moving dim) so the expert-count AllReduce can launch early enough
# to hide under the tail of the output writes.
SEGMENTS = ((0, 1), (1, 1), (2, 2))

_cached = {}


def _build_nc():
    nc = bacc.Bacc("TRN2", target_bir_lowering=False, num_devices=NCORES)

    x_d = nc.dram_tensor("x", [NT, H], DT, kind="ExternalInput")
    w1_d = nc.dram_tensor("w1", [H, H], DT, kind="ExternalInput")
    w2_d = nc.dram_tensor("w2m", [P, KC * E], DT, kind="ExternalInput")
    b1_d = nc.dram_tensor("b1m", [P, KC], DT, kind="ExternalInput")
    b2_d = nc.dram_tensor("b2m", [E, 1], DT, kind="ExternalInput")

    disp_d = nc.dram_tensor("disp", [NT, E, CAPACITY], DT, kind="ExternalOutput")
    comb_d = nc.dram_tensor("comb", [NT, E, CAPACITY], DT, kind="ExternalOutput")
    probs_d = nc.dram_tensor("probs", [NT, E], DT, kind="ExternalOutput")
    aux_d = nc.dram_tensor("aux", [1, 1], DT, kind="ExternalOutput")

    cc_in = nc.dram_tensor("cc_in", [1, 2 * E], DT)
    cc_out = nc.dram_tensor("cc_out", [1, 2 * E], DT, addr_space="Shared")

    AF = mybir.ActivationFunctionType
    ALU = mybir.AluOpType
    AX = mybir.AxisListType

    with tile.TileContext(nc) as tc:
        with (
            tc.tile_pool(name="const", bufs=1) as cpool,
            tc.tile_pool(name="work", bufs=2) as wpool,
            tc.tile_pool(name="rows", bufs=2) as rpool,
            tc.tile_pool(name="ph", bufs=1, space="PSUM") as p_h,
            tc.tile_pool(name="pl", bufs=1, space="PSUM") as p_l,
            tc.tile_pool(name="ptr", bufs=1, space="PSUM") as p_tr,
            tc.tile_pool(name="pxt", bufs=1, space="PSUM") as p_xt,
            tc.tile_pool(name="pred", bufs=1, space="PSUM") as p_red,
        ):
            # ---- constants ----
            ident = cpool.tile([P, P], DT, tag="ident")
            make_identity(nc, ident[:])
            ones = cpool.tile([P, 1], DT, tag="ones")
            nc.vector.memset(ones[:], 1.0)

            # Zero both row-staging slots per output up front (25us of DVE
            # memset that must not sit between compute and the first row
            # DMA). Later tiles reuse these slots; only the 8 live columns
            # are ever rewritten, the rest stays zero.
            HE = E // 2
            for _ in range(2):
                zr = rpool.tile([P, HE, CAPACITY], DT, tag="dispr", name="zr")
                nc.gpsimd.memset(zr[:], 0.0)
                zc = rpool.tile([P, HE, CAPACITY], DT, tag="combr", name="zc")
                nc.scalar.memzero(zc[:])

            # ---- load inputs: W1 split across both HWDGE queues (x0 ahead
            # of the odd chunks so block 0's transposes start early) ----
            x_sb = [
                cpool.tile([P, H], DT, tag=f"x_{t}", name=f"x_{t}")
                for t in range(TBLK)
            ]
            w1_sb = [
                cpool.tile([P, H], DT, tag=f"w1_{k}", name=f"w1_{k}")
                for k in range(KC)
            ]
            nc.sync.dma_start(x_sb[0][:], x_d[0:P, :])
            w1_eng = (nc.sync, nc.scalar, nc.gpsimd)
            for k in range(KC):
                w1_eng[k % 3].dma_start(w1_sb[k][:], w1_d[k * P:(k + 1) * P, :])
            nc.scalar.dma_start(x_sb[1][:], x_d[P:2 * P, :])
            nc.sync.dma_start(x_sb[2][:], x_d[2 * P:3 * P, :])
            nc.scalar.dma_start(x_sb[3][:], x_d[3 * P:4 * P, :])
            w2_sb = cpool.tile([P, KC * E], DT, tag="w2")
            nc.scalar.dma_start(w2_sb[:], w2_d[:])
            b1_sb = cpool.tile([P, KC], DT, tag="b1")
            nc.scalar.dma_start(b1_sb[:], b1_d[:])
            b2_sb = cpool.tile([E, 1], DT, tag="b2")
            nc.scalar.dma_start(b2_sb[:], b2_d[:])

            # ---- transpose x: [tok, H] -> xT chunks [128h, NT], block-major
            # so block 0's slices are ready first; the psum pool closes so
            # its banks return to the free pool before the MLP needs them ----
            xT_sb = [
                cpool.tile([P, NT], DT, tag=f"xT_{k}", name=f"xT_{k}")
                for k in range(KC)
            ]
            for t in range(TBLK):
                for k in range(KC):
                    pt = (p_xt.tile([P, P], DT, tag="xtp", name="pt")
                          if k % 2 == 0 else
                          p_tr.tile([P, P], DT, tag="trp", name="pt"))
                    nc.tensor.transpose(
                        pt[:], x_sb[t][:, k * P:(k + 1) * P], ident[:],
                    )
                    nc.vector.tensor_copy(
                        xT_sb[k][:, t * P:(t + 1) * P], pt[:],
                    )

            p_red2 = p_red.tile([1, 2 * E], DT, tag="cntps", name="p_red2")
            n_done = 0

            # ---- per-segment compute, per-block row writes ----
            for seg_start, seg_n in SEGMENTS:
                W = seg_n * P
                ts0 = seg_start * P

                # matmul1: hT [c-chunk, W tokens]; 8 sequential accumulation
                # groups split across two psum tiles.
                ph0 = p_h.tile([P, 4 * W], DT, tag="hp0")
                ph1 = p_h.tile([P, 4 * W], DT, tag="hp1")
                phs = (ph0, ph1)
                for c in range(KC):
                    for k in range(KC):
                        nc.tensor.matmul(
                            phs[c // 4][:, (c % 4) * W:(c % 4 + 1) * W],
                            w1_sb[k][:, c * P:(c + 1) * P],
                            xT_sb[k][:, ts0:ts0 + W],
                            start=(k == 0),
                            stop=(k == KC - 1),
                            skip_group_check=True,
                        )
                # bias + relu evac
                hT_t = []
                for c in range(KC):
                    ht = wpool.tile([P, W], DT, tag=f"hT_{c}")
                    nc.vector.tensor_scalar(
                        ht[:], phs[c // 4][:, (c % 4) * W:(c % 4 + 1) * W],
                        b1_sb[:, c:c + 1], 0.0, op0=ALU.add, op1=ALU.max,
                    )
                    hT_t.append(ht)

                # matmul2: logitsT [E, W]
                pl = p_l.tile([E, W], DT, tag="lp")
                for k in range(KC):
                    nc.tensor.matmul(
                        pl[:],
                        w2_sb[:, k * E:(k + 1) * E],
                        hT_t[k][:],
                        start=(k == 0),
                        stop=(k == KC - 1),
                    )
                lts = wpool.tile([E, W], DT, tag="lts")
                nc.vector.tensor_scalar(
                    lts[:], pl[:], b2_sb[:, 0:1], None, op0=ALU.add,
                )

                for sub in range(seg_n):
                    t = seg_start + sub
                    # transpose to [128 tok, E]; softmax along E
                    ptr = p_tr.tile([P, E], DT, tag="trp")
                    nc.tensor.transpose(
                        ptr[:], lts[:, sub * P:(sub + 1) * P], ident[0:E, 0:E],
                    )
                    negmax = wpool.tile([P, 1], DT, tag="negmax")
                    nc.vector.tensor_reduce(
                        out=negmax[:], in_=ptr[:], axis=AX.X, op=ALU.max,
                        negate=True,
                    )
                    ex = wpool.tile([P, E], DT, tag="ex")
                    sume = wpool.tile([P, 1], DT, tag="sume")
                    nc.scalar.activation(
                        ex[:], ptr[:], AF.Exp,
                        bias=negmax[:], scale=1.0, accum_out=sume[:],
                    )
                    rec = wpool.tile([P, 1], DT, tag="rec")
                    nc.vector.reciprocal(rec[:], sume[:])
                    ohpr = wpool.tile([P, 2 * E], DT, tag="ohpr")
                    oh0 = ohpr[:, 0:E]
                    pr = ohpr[:, E:2 * E]
                    nc.vector.tensor_scalar_mul(pr, ex[:], rec[:, 0:1])
                    nc.gpsimd.dma_start(probs_d[t * P:(t + 1) * P, :], pr)

                    # top-2 via the DVE top-8 sorter; onehots by exact match
                    mx = wpool.tile([P, E], DT, tag="mx")
                    nc.vector.max(out=mx[:], in_=pr)
                    nc.vector.tensor_scalar(
                        oh0, pr, mx[:, 0:1], None, op0=ALU.is_equal,
                    )
                    oh1 = wpool.tile([P, E], DT, tag="oh1")
                    nc.vector.tensor_scalar(
                        oh1[:], pr, mx[:, 1:2], None, op0=ALU.is_equal,
                    )
                    den = wpool.tile([P, 1], DT, tag="den")
                    nc.vector.tensor_add(den[:], mx[:, 0:1], mx[:, 1:2])
                    nrec = wpool.tile([P, 1], DT, tag="nrec")
                    nc.vector.reciprocal(nrec[:], den[:])
                    pn0 = wpool.tile([P, 1], DT, tag="pn0")
                    nc.vector.tensor_mul(pn0[:], mx[:, 0:1], nrec[:])
                    pn1 = wpool.tile([P, 1], DT, tag="pn1")
                    nc.vector.tensor_mul(pn1[:], mx[:, 1:2], nrec[:])
                    s0 = wpool.tile([P, E], DT, tag="s0")
                    nc.vector.tensor_scalar_mul(s0[:], oh0, pn0[:, 0:1])
                    s1 = wpool.tile([P, E], DT, tag="s1")
                    nc.vector.tensor_scalar_mul(s1[:], oh1[:], pn1[:, 0:1])

                    # partition reduction: [top-1 counts | prob sums] in one
                    # accumulation group
                    nc.tensor.matmul(
                        p_red2[:], ones[:], ohpr[:],
                        start=(n_done == 0), stop=(n_done == TBLK - 1),
                        skip_group_check=True,
                    )
                    if n_done == TBLK - 1:
                        # AllReduce [count | prob_sum]: evacuate on the
                        # scalar engine (its stream is idle here; the vector
                        # stream convoys behind row-DMA waits) and bounce
                        # through the software DGE so the 64B transfer does
                        # not queue behind megabyte row writes.
                        ccin_sb = cpool.tile([1, 2 * E], DT, tag="ccin")
                        nc.scalar.copy(ccin_sb[:], p_red2[:])
                        nc.gpsimd.dma_start(cc_in[:], ccin_sb[:])
                        nc.gpsimd.collective_compute(
                            "AllReduce",
                            ALU.add,
                            replica_groups=[list(range(NCORES))],
                            ins=[cc_in[:]],
                            outs=[cc_out[:]],
                        )

                    # Predicted rows (appeared[e] == 1 for all e): slot0 <-
                    # top1, slot1 <- top2; host-guarded for the degenerate
                    # case. Half-rows (4 experts) double-buffer in the same
                    # SBUF footprint as full rows, so the column writes never
                    # stall on the previous row DMA. Every half-row slot has
                    # the same local layout (columns [e_local, 0:2] live,
                    # rest zero), so slot rotation is safe.
                    for hh in range(2):
                        es = slice(hh * HE, (hh + 1) * HE)
                        drow = rpool.tile([P, HE, CAPACITY], DT, tag="dispr")
                        nc.vector.tensor_copy(drow[:, :, 0], oh0[:, es])
                        nc.vector.tensor_copy(drow[:, :, 1], oh1[:, es])
                        nc.sync.dma_start(
                            disp_d[t * P:(t + 1) * P, es, :], drow[:],
                        )
                        crow = rpool.tile([P, HE, CAPACITY], DT, tag="combr")
                        nc.vector.tensor_copy(crow[:, :, 0], s0[:, es])
                        nc.vector.tensor_copy(crow[:, :, 1], s1[:, es])
                        nc.scalar.dma_start(
                            comb_d[t * P:(t + 1) * P, es, :], crow[:],
                        )
                    n_done += 1

            # ---- aux loss from the AllReduce result ----
            cc_sb = cpool.tile([1, 2 * E], DT, tag="ccout")
            nc.gpsimd.dma_start(cc_sb[:], cc_out[:])
            # aux = sum_e m_e * log(m_e * E + 1e-9), m = prob_sum / N
            pe8 = cpool.tile([1, E], DT, tag="pe8")
            nc.vector.tensor_scalar(
                pe8[:], cc_sb[:, E:2 * E], float(E) / N, 1e-9,
                op0=ALU.mult, op1=ALU.add,
            )
            lg = cpool.tile([1, E], DT, tag="lg")
            nc.scalar.activation(lg[:], pe8[:], AF.Ln)
            pe = cpool.tile([1, E], DT, tag="pe")
            nc.vector.tensor_scalar_mul(pe[:], cc_sb[:, E:2 * E], 1.0 / N)
            prod = cpool.tile([1, E], DT, tag="prod")
            nc.vector.tensor_mul(prod[:], pe[:], lg[:])
            aux_sb = cpool.tile([1, 1], DT, tag="aux")
            nc.vector.tensor_reduce(
                out=aux_sb[:], in_=prod[:], axis=AX.X, op=ALU.add,
            )
            nc.gpsimd.dma_start(aux_d[:], aux_sb[:])

    nc.compile()
    return nc


def kernel(hidden_states, W1, b1, W2, b2):
    hidden_states = np.ascontiguousarray(hidden_states, dtype=np.float32)
    W1 = np.ascontiguousarray(W1, dtype=np.float32)
    b1 = np.asarray(b1, dtype=np.float32)
    W2 = np.asarray(W2, dtype=np.float32)
    b2 = np.asarray(b2, dtype=np.float32)

    if "nc" not in _cached:
        _cached["nc"] = _build_nc()
    nc = _cached["nc"]

    x = hidden_states.reshape(N, H)
    # host-side marshalling of the (replicated) small weights into the
    # layouts the kernel consumes
    b1m = np.ascontiguousarray(b1.reshape(KC, P).T)            # [128, 8]
    w2m = np.ascontiguousarray(
        W2.reshape(KC, P, E).transpose(1, 0, 2).reshape(P, KC * E)
    )                                                          # [128, 64]
    b2m = np.ascontiguousarray(b2.reshape(E, 1))               # [8, 1]

    in_maps = []
    for c in range(NCORES):
        in_maps.append({
            "x": np.ascontiguousarray(x[c * NT:(c + 1) * NT]),
            "w1": W1,
            "w2m": w2m,
            "b1m": b1m,
            "b2m": b2m,
        })

    res = run_bass_kernel_spmd(nc, in_maps, core_ids=list(range(NCORES)))
    _cached["last_result"] = res

    dispatch = np.concatenate(
        [r["disp"] for r in res.results], axis=0
    ).reshape(B, S, E, CAPACITY)
    combine = np.concatenate(
        [r["comb"] for r in res.results], axis=0
    ).reshape(B, S, E, CAPACITY)
    probs = np.concatenate(
        [r["probs"] for r in res.results], axis=0
    ).reshape(B, S, E)
    aux_loss = np.float32(res.results[0]["aux"][0, 0])

    # The device writes dispatch/combine assuming every expert is someone's
    # top-1 (true for any realistic routing batch; verified exact against
    # the reference). Guard the degenerate case where some expert is
    # globally unused: rebuild the two live capacity columns on host from
    # the device-computed probs, so slot-1 placement matches the reference
    # count semantics.
    flat_probs = probs.reshape(N, E)
    i0 = flat_probs.argmax(-1)
    appeared = np.zeros(E, bool)
    appeared[i0] = True
    if not appeared.all():
        ar = np.arange(N)
        pm = flat_probs.copy()
        pm[ar, i0] = -1.0
        i1 = pm.argmax(-1)
        p0 = flat_probs[ar, i0]
        p1 = flat_probs[ar, i1]
        pn0 = p0 / (p0 + p1)
        pn1 = p1 / (p0 + p1)
        a = appeared.astype(np.float32)
        oh0 = np.zeros((N, E), np.float32)
        oh0[ar, i0] = 1.0
        oh1 = np.zeros((N, E), np.float32)
        oh1[ar, i1] = 1.0
        dflat = dispatch.reshape(N, E, CAPACITY)
        cflat = combine.reshape(N, E, CAPACITY)
        dflat[:, :, 0] = oh0 + oh1 * (1.0 - a)
        dflat[:, :, 1] = oh1 * a
        cflat[:, :, 0] = pn0[:, None] * oh0 + pn1[:, None] * oh1 * (1.0 - a)
        cflat[:, :, 1] = pn1[:, None] * oh1 * a

    return dispatch, combine, probs, aux_loss


# revision 23
# speedup vs baseline: 1.0704x; 1.0704x over previous
"""Trainium2 Bass kernel for MoE router (BaseRouter): 8-core data-parallel.

Reference computation (per token): router MLP (Linear-ReLU-Linear) -> softmax
-> top-2 -> dispatch/combine one-hot tensors [N, E, CAPACITY] + aux load loss.

Sharding: tokens (B*S = 4096) split 512/core across 8 cores; router weights
replicated. The dispatch/combine capacity index is always 0 for a token's
top-1 expert and `appeared[e]` (0/1) for its top-2 expert, where appeared[e]
says whether ANY token globally picked e as top-1.

The kernel is output-write bound (2 x 25 MiB of mostly-zero rows per core),
so the structure optimizes time-to-first-write and write bandwidth:
 - outputs are staged as half-rows (4 experts x 1536 capacity) in SBUF:
   24 KiB DMA packets, double-buffered, only the 8 columns [e, 0:2] are
   ever rewritten after a one-time memset; dispatch rides the sync HWDGE
   queue and combine the scalar one;
 - compute is pipelined per token segment so the first row DMA issues after
   ~1/5 of the MLP; later blocks share matmul weight loads (wider moving
   dim) since the write wall, not the PE, paces them;
 - rows are written assuming appeared[e] == 1 (true unless some expert is
   nobody's top-1); the 8-core AllReduce (expert counts fused with the
   aux-loss prob sums) runs off the critical path, overlapped with the
   remaining output writes. The degenerate appeared[e] == 0 case is
   patched during the host-side unshard from the (always exact) probs.
"""

import sys

if "/opt/trn_rl_repo" not in sys.path:
    sys.path.insert(0, "/opt/trn_rl_repo")

import numpy as np

import concourse.bass as bass  # noqa: F401  (engine types referenced via nc)
import concourse.mybir as mybir
import concourse.tile as tile
from concourse import bacc
from concourse.bass_utils import run_bass_kernel_spmd
from concourse.masks import make_identity

B, S, H, E, TOPK = 2, 2048, 1024, 8, 2
CAPACITY = 1536
N = B * S                  # 4096 tokens
NCORES = 8
NT = N // NCORES           # 512 tokens per core
P = 128                    # SBUF partitions
TBLK = NT // P             # 4 token tiles per core
KC = H // P                # 8 contraction chunks
DT = mybir.dt.float32
# (start_block, n_blocks) compute segments: early blocks solo to open the
# write pipeline ASAP, later blocks fused (weight loads amortized over a
# wider moving dim) so the expert-count AllReduce can launch early enough
# to hide under the tail of the output writes.
SEGMENTS = ((0, 1), (1, 1), (2, 2))

_cached = {}


def _build_nc():
    nc = bacc.Bacc("TRN2", target_bir_lowering=False, num_devices=NCORES)

    x_d = nc.dram_tensor("x", [NT, H], DT, kind="ExternalInput")
    w1_d = nc.dram_tensor("w1", [H, H], DT, kind="ExternalInput")
    w2_d = nc.dram_tensor("w2m", [P, KC * E], DT, kind="ExternalInput")
    b1_d = nc.dram_tensor("b1m", [P, KC], DT, kind="ExternalInput")
    b2_d = nc.dram_tensor("b2m", [E, 1], DT, kind="ExternalInput")

    disp_d = nc.dram_tensor("disp", [NT, E, CAPACITY], DT, kind="ExternalOutput")
    comb_d = nc.dram_tensor("comb", [NT, E, CAPACITY], DT, kind="ExternalOutput")
    probs_d = nc.dram_tensor("probs", [NT, E], DT, kind="ExternalOutput")
    aux_d = nc.dram_tensor("aux", [1, 1], DT, kind="ExternalOutput")

    cc_in = nc.dram_tensor("cc_in", [1, 2 * E], DT)
    cc_out = nc.dram_tensor("cc_out", [1, 2 * E], DT, addr_space="Shared")
    cc_warm_in = nc.dram_tensor("cc_warm_in", [1, 2 * E], DT)
    cc_warm_out = nc.dram_tensor("cc_warm_out", [1, 2 * E], DT, addr_space="Shared")

    AF = mybir.ActivationFunctionType
    ALU = mybir.AluOpType
    AX = mybir.AxisListType

    with tile.TileContext(nc) as tc:
        with (
            tc.tile_pool(name="const", bufs=1) as cpool,
            tc.tile_pool(name="work", bufs=2) as wpool,
            tc.tile_pool(name="rows", bufs=2) as rpool,
            tc.tile_pool(name="ph", bufs=1, space="PSUM") as p_h,
            tc.tile_pool(name="pl", bufs=1, space="PSUM") as p_l,
            tc.tile_pool(name="ptr", bufs=1, space="PSUM") as p_tr,
            tc.tile_pool(name="pxt", bufs=1, space="PSUM") as p_xt,
            tc.tile_pool(name="pred", bufs=1, space="PSUM") as p_red,
        ):
            # ---- constants ----
            ident = cpool.tile([P, P], DT, tag="ident")
            make_identity(nc, ident[:])
            ones = cpool.tile([P, 1], DT, tag="ones")
            nc.vector.memset(ones[:], 1.0)

            # Zero both row-staging slots per output up front (25us of DVE
            # memset that must not sit between compute and the first row
            # DMA). Later tiles reuse these slots; only the 8 live columns
            # are ever rewritten, the rest stays zero.
            HE = E // 2
            for _ in range(2):
                zr = rpool.tile([P, HE, CAPACITY], DT, tag="dispr", name="zr")
                nc.gpsimd.memset(zr[:], 0.0)
                zc = rpool.tile([P, HE, CAPACITY], DT, tag="combr", name="zc")
                nc.scalar.memzero(zc[:])

            # ---- load inputs: W1 split across both HWDGE queues (x0 ahead
            # of the odd chunks so block 0's transposes start early) ----
            x_sb = [
                cpool.tile([P, H], DT, tag=f"x_{t}", name=f"x_{t}")
                for t in range(TBLK)
            ]
            w1_sb = [
                cpool.tile([P, H], DT, tag=f"w1_{k}", name=f"w1_{k}")
                for k in range(KC)
            ]
            nc.sync.dma_start(x_sb[0][:], x_d[0:P, :])
            w1_eng = (nc.sync, nc.scalar, nc.gpsimd)
            for k in range(KC):
                w1_eng[k % 3].dma_start(w1_sb[k][:], w1_d[k * P:(k + 1) * P, :])
            nc.scalar.dma_start(x_sb[1][:], x_d[P:2 * P, :])
            nc.sync.dma_start(x_sb[2][:], x_d[2 * P:3 * P, :])
            nc.scalar.dma_start(x_sb[3][:], x_d[3 * P:4 * P, :])
            w2_sb = cpool.tile([P, KC * E], DT, tag="w2")
            nc.scalar.dma_start(w2_sb[:], w2_d[:])
            b1_sb = cpool.tile([P, KC], DT, tag="b1")
            nc.scalar.dma_start(b1_sb[:], b1_d[:])
            b2_sb = cpool.tile([E, 1], DT, tag="b2")
            nc.scalar.dma_start(b2_sb[:], b2_d[:])

            # Warm up the collective stack while compute starts: the first
            # AllReduce on a NEFF pays ncfw setup cost; this dummy one runs
            # during the write stream so the real (tiny) one later is fast.
            ccw_sb = cpool.tile([1, 2 * E], DT, tag="ccw")
            nc.vector.memset(ccw_sb[:], 0.0)
            nc.gpsimd.dma_start(cc_warm_in[:], ccw_sb[:])
            nc.gpsimd.collective_compute(
                "AllReduce",
                ALU.add,
                replica_groups=[list(range(NCORES))],
                ins=[cc_warm_in[:]],
                outs=[cc_warm_out[:]],
            )

            # ---- transpose x: [tok, H] -> xT chunks [128h, NT], block-major
            # so block 0's slices are ready first; the psum pool closes so
            # its banks return to the free pool before the MLP needs them ----
            xT_sb = [
                cpool.tile([P, NT], DT, tag=f"xT_{k}", name=f"xT_{k}")
                for k in range(KC)
            ]
            for t in range(TBLK):
                for k in range(KC):
                    pt = (p_xt.tile([P, P], DT, tag="xtp", name="pt")
                          if k % 2 == 0 else
                          p_tr.tile([P, P], DT, tag="trp", name="pt"))
                    nc.tensor.transpose(
                        pt[:], x_sb[t][:, k * P:(k + 1) * P], ident[:],
                    )
                    nc.vector.tensor_copy(
                        xT_sb[k][:, t * P:(t + 1) * P], pt[:],
                    )

            p_red2 = p_red.tile([1, 2 * E], DT, tag="cntps", name="p_red2")
            n_done = 0

            # ---- per-segment compute, per-block row writes ----
            for seg_start, seg_n in SEGMENTS:
                W = seg_n * P
                ts0 = seg_start * P

                # matmul1: hT [c-chunk, W tokens]; 8 sequential accumulation
                # groups split across two psum tiles.
                ph0 = p_h.tile([P, 4 * W], DT, tag="hp0")
                ph1 = p_h.tile([P, 4 * W], DT, tag="hp1")
                phs = (ph0, ph1)
                for c in range(KC):
                    for k in range(KC):
                        nc.tensor.matmul(
                            phs[c // 4][:, (c % 4) * W:(c % 4 + 1) * W],
                            w1_sb[k][:, c * P:(c + 1) * P],
                            xT_sb[k][:, ts0:ts0 + W],
                            start=(k == 0),
                            stop=(k == KC - 1),
                            skip_group_check=True,
                        )
                # bias + relu evac
                hT_t = []
                for c in range(KC):
                    ht = wpool.tile([P, W], DT, tag=f"hT_{c}")
                    nc.vector.tensor_scalar(
                        ht[:], phs[c // 4][:, (c % 4) * W:(c % 4 + 1) * W],
                        b1_sb[:, c:c + 1], 0.0, op0=ALU.add, op1=ALU.max,
                    )
                    hT_t.append(ht)

                # matmul2: logitsT [E, W]
                pl = p_l.tile([E, W], DT, tag="lp")
                for k in range(KC):
                    nc.tensor.matmul(
                        pl[:],
                        w2_sb[:, k * E:(k + 1) * E],
                        hT_t[k][:],
                        start=(k == 0),
                        stop=(k == KC - 1),
                    )
                lts = wpool.tile([E, W], DT, tag="lts")
                nc.vector.tensor_scalar(
                    lts[:], pl[:], b2_sb[:, 0:1], None, op0=ALU.add,
                )

                for sub in range(seg_n):
                    t = seg_start + sub
                    # transpose to [128 tok, E]; softmax along E
                    ptr = p_tr.tile([P, E], DT, tag="trp")
                    nc.tensor.transpose(
                        ptr[:], lts[:, sub * P:(sub + 1) * P], ident[0:E, 0:E],
                    )
                    negmax = wpool.tile([P, 1], DT, tag="negmax")
                    nc.vector.tensor_reduce(
                        out=negmax[:], in_=ptr[:], axis=AX.X, op=ALU.max,
                        negate=True,
                    )
                    ex = wpool.tile([P, E], DT, tag="ex")
                    sume = wpool.tile([P, 1], DT, tag="sume")
                    nc.scalar.activation(
                        ex[:], ptr[:], AF.Exp,
                        bias=negmax[:], scale=1.0, accum_out=sume[:],
                    )
                    rec = wpool.tile([P, 1], DT, tag="rec")
                    nc.vector.reciprocal(rec[:], sume[:])
                    ohpr = wpool.tile([P, 2 * E], DT, tag="ohpr")
                    oh0 = ohpr[:, 0:E]
                    pr = ohpr[:, E:2 * E]
                    nc.vector.tensor_scalar_mul(pr, ex[:], rec[:, 0:1])
                    nc.gpsimd.dma_start(probs_d[t * P:(t + 1) * P, :], pr)

                    # top-2 via the DVE top-8 sorter; onehots by exact match
                    mx = wpool.tile([P, E], DT, tag="mx")
                    nc.vector.max(out=mx[:], in_=pr)
                    nc.vector.tensor_scalar(
                        oh0, pr, mx[:, 0:1], None, op0=ALU.is_equal,
                    )
                    oh1 = wpool.tile([P, E], DT, tag="oh1")
                    nc.vector.tensor_scalar(
                        oh1[:], pr, mx[:, 1:2], None, op0=ALU.is_equal,
                    )
                    den = wpool.tile([P, 1], DT, tag="den")
                    nc.vector.tensor_add(den[:], mx[:, 0:1], mx[:, 1:2])
                    nrec = wpool.tile([P, 1], DT, tag="nrec")
                    nc.vector.reciprocal(nrec[:], den[:])
                    pn0 = wpool.tile([P, 1], DT, tag="pn0")
                    nc.vector.tensor_mul(pn0[:], mx[:, 0:1], nrec[:])
                    pn1 = wpool.tile([P, 1], DT, tag="pn1")
                    nc.vector.tensor_mul(pn1[:], mx[:, 1:2], nrec[:])
                    s0 = wpool.tile([P, E], DT, tag="s0")
                    nc.vector.tensor_scalar_mul(s0[:], oh0, pn0[:, 0:1])
                    s1 = wpool.tile([P, E], DT, tag="s1")
                    nc.vector.tensor_scalar_mul(s1[:], oh1[:], pn1[:, 0:1])

                    # partition reduction: [top-1 counts | prob sums] in one
                    # accumulation group
                    nc.tensor.matmul(
                        p_red2[:], ones[:], ohpr[:],
                        start=(n_done == 0), stop=(n_done == TBLK - 1),
                        skip_group_check=True,
                    )
                    if n_done == TBLK - 1:
                        # AllReduce [count | prob_sum]: evacuate on the
                        # scalar engine (its stream is idle here; the vector
                        # stream convoys behind row-DMA waits) and bounce
                        # through the software DGE so the 64B transfer does
                        # not queue behind megabyte row writes.
                        ccin_sb = cpool.tile([1, 2 * E], DT, tag="ccin")
                        nc.scalar.copy(ccin_sb[:], p_red2[:])
                        nc.gpsimd.dma_start(cc_in[:], ccin_sb[:])
                        nc.gpsimd.collective_compute(
                            "AllReduce",
                            ALU.add,
                            replica_groups=[list(range(NCORES))],
                            ins=[cc_in[:]],
                            outs=[cc_out[:]],
                        )

                    # Predicted rows (appeared[e] == 1 for all e): slot0 <-
                    # top1, slot1 <- top2; host-guarded for the degenerate
                    # case. Half-rows (4 experts) double-buffer in the same
                    # SBUF footprint as full rows, so the column writes never
                    # stall on the previous row DMA. Every half-row slot has
                    # the same local layout (columns [e_local, 0:2] live,
                    # rest zero), so slot rotation is safe.
                    for hh in range(2):
                        es = slice(hh * HE, (hh + 1) * HE)
                        drow = rpool.tile([P, HE, CAPACITY], DT, tag="dispr")
                        nc.vector.tensor_copy(drow[:, :, 0], oh0[:, es])
                        nc.vector.tensor_copy(drow[:, :, 1], oh1[:, es])
                        nc.sync.dma_start(
                            disp_d[t * P:(t + 1) * P, es, :], drow[:],
                        )
                        crow = rpool.tile([P, HE, CAPACITY], DT, tag="combr")
                        nc.vector.tensor_copy(crow[:, :, 0], s0[:, es])
                        nc.vector.tensor_copy(crow[:, :, 1], s1[:, es])
                        nc.scalar.dma_start(
                            comb_d[t * P:(t + 1) * P, es, :], crow[:],
                        )
                    n_done += 1

            # ---- aux loss from the AllReduce result ----
            cc_sb = cpool.tile([1, 2 * E], DT, tag="ccout")
            nc.gpsimd.dma_start(cc_sb[:], cc_out[:])
            # aux = sum_e m_e * log(m_e * E + 1e-9), m = prob_sum / N
            pe8 = cpool.tile([1, E], DT, tag="pe8")
            nc.vector.tensor_scalar(
                pe8[:], cc_sb[:, E:2 * E], float(E) / N, 1e-9,
                op0=ALU.mult, op1=ALU.add,
            )
            lg = cpool.tile([1, E], DT, tag="lg")
            nc.scalar.activation(lg[:], pe8[:], AF.Ln)
            pe = cpool.tile([1, E], DT, tag="pe")
            nc.vector.tensor_scalar_mul(pe[:], cc_sb[:, E:2 * E], 1.0 / N)
            prod = cpool.tile([1, E], DT, tag="prod")
            nc.vector.tensor_mul(prod[:], pe[:], lg[:])
            aux_sb = cpool.tile([1, 1], DT, tag="aux")
            nc.vector.tensor_reduce(
                out=aux_sb[:], in_=prod[:], axis=AX.X, op=ALU.add,
            )
            nc.gpsimd.dma_start(aux_d[:], aux_sb[:])

    nc.compile()
    return nc


def kernel(hidden_states, W1, b1, W2, b2):
    hidden_states = np.ascontiguousarray(hidden_states, dtype=np.float32)
    W1 = np.ascontiguousarray(W1, dtype=np.float32)
    b1 = np.asarray(b1, dtype=np.float32)
    W2 = np.asarray(W2, dtype=np.float32)
    b2 = np.asarray(b2, dtype=np.float32)

    if "nc" not in _cached:
        _cached["nc"] = _build_nc()
    nc = _cached["nc"]

    x = hidden_states.reshape(N, H)
    # host-side marshalling of the (replicated) small weights into the
    # layouts the kernel consumes
    b1m = np.ascontiguousarray(b1.reshape(KC, P).T)            # [128, 8]
    w2m = np.ascontiguousarray(
        W2.reshape(KC, P, E).transpose(1, 0, 2).reshape(P, KC * E)
    )                                                          # [128, 64]
    b2m = np.ascontiguousarray(b2.reshape(E, 1))               # [8, 1]

    in_maps = []
    for c in range(NCORES):
        in_maps.append({
            "x": np.ascontiguousarray(x[c * NT:(c + 1) * NT]),
            "w1": W1,
            "w2m": w2m,
            "b1m": b1m,
            "b2m": b2m,
        })

    try:
        res = run_bass_kernel_spmd(nc, in_maps, core_ids=list(range(NCORES)))
    except Exception:
        # transient NRT exec-unit errors have been observed on a cold
        # device; one retry has always recovered
        res = run_bass_kernel_spmd(nc, in_maps, core_ids=list(range(NCORES)))
    _cached["last_result"] = res

    dispatch = np.concatenate(
        [r["disp"] for r in res.results], axis=0
    ).reshape(B, S, E, CAPACITY)
    combine = np.concatenate(
        [r["comb"] for r in res.results], axis=0
    ).reshape(B, S, E, CAPACITY)
    probs = np.concatenate(
        [r["probs"] for r in res.results], axis=0
    ).reshape(B, S, E)
    aux_loss = np.float32(res.results[0]["aux"][0, 0])

    # The device writes dispatch/combine assuming every expert is someone's
    # top-1 (true for any realistic routing batch; verified exact against
    # the reference). Guard the degenerate case where some expert is
    # globally unused: rebuild the two live capacity columns on host from
    # the device-computed probs, so slot-1 placement matches the reference
    # count semantics.
    flat_probs = probs.reshape(N, E)
    i0 = flat_probs.argmax(-1)
    appeared = np.zeros(E, bool)
    appeared[i0] = True
    if not appeared.all():
        ar = np.arange(N)
        pm = flat_probs.copy()
        pm[ar, i0] = -1.0
        i1 = pm.argmax(-1)
        p0 = flat_probs[ar, i0]
        p1 = flat_probs[ar, i1]
        pn0 = p0 / (p0 + p1)
        pn1 = p1 / (p0 + p1)
        a = appeared.astype(np.float32)
        oh0 = np.zeros((N, E), np.float32)
        oh0[ar, i0] = 1.0
        oh1 = np.zeros((N, E), np.float32)
        oh1[ar, i1] = 1.0
        dflat = dispatch.reshape(N, E, CAPACITY)
        cflat = combine.reshape(N, E, CAPACITY)
        dflat[:, :, 0] = oh0 + oh1 * (1.0 - a)
        dflat[:, :, 1] = oh1 * a
        cflat[:, :, 0] = pn0[:, None] * oh0 + pn1[:, None] * oh1 * (1.0 - a)
        cflat[:, :, 1] = pn1[:, None] * oh1 * a

    return dispatch, combine, probs, aux_loss
